# revision 1
# baseline (speedup 1.0000x reference)
"""Multi-head causal attention (B=2, L=2048, H=2048, NH=16) on 8 Trainium2
NeuronCores.

Sharding: tensor-parallel over heads — core c computes heads {2c, 2c+1}.
Each core:
  phase 1: q/k/v projections for its 256 output dims (contract over H=2048)
  phase 2: causal attention for its 2 heads + its partial o-projection
Host: transposes/rounds inputs (fp32r layout prep), sums the 8 partial
o-projection outputs, and transposes back.

All matmuls run in float32r (fp32 with 11-bit mantissa, 1 cycle/row on the
PE for free dims >= 256 — 4x faster than plain fp32 at ~2.4e-4 rounding).
Causal masking skips fully-masked j-tiles (halves attention FLOPs) and
zeroes the diagonal tiles post-exp with gpsimd.affine_select.
"""

import os
import sys

if "/opt/trn_rl_repo" not in sys.path:
    sys.path.insert(0, "/opt/trn_rl_repo")

import numpy as np

from concourse import bacc, mybir, tile  # noqa: E402
from concourse.bass_utils import run_bass_kernel_spmd  # noqa: E402

F32R = mybir.dt.float32r
F32 = mybir.dt.float32

N_CORES = 8
B, L, H, NH = 2, 2048, 2048, 16
DH = H // NH                       # 128
BL = B * L                        # 4096
HPC = NH // N_CORES               # heads per core = 2
OPC = HPC * DH                    # output dims per core = 256
HT = H // 128                     # 16 h-tiles (contraction)
IC1 = 256                         # phase-1 i-chunk width
N_IC1 = BL // IC1                 # 16
IC2 = 512                         # phase-2 i-chunk width
N_IC2 = L // IC2                  # 4 per batch
JT = L // 128                     # 16 j-tiles per batch
SCALE = 1.0 / float(np.sqrt(DH))

LAST_EXEC_NS = None


def _round_fp32r(a: np.ndarray) -> np.ndarray:
    """Round fp32 to fp32r (11-bit mantissa, round-to-nearest-even)."""
    a = np.ascontiguousarray(a, dtype=np.float32)
    u = a.view(np.uint32)
    low = u & np.uint32(0xFFF)
    rounded = (u & np.uint32(0xFFFFF000)).astype(np.uint64)
    half = np.uint32(0x800)
    lsb = (u >> np.uint32(12)) & np.uint32(1)
    up = (low > half) | ((low == half) & (lsb == 1))
    rounded = rounded + (up.astype(np.uint64) << np.uint64(12))
    return rounded.astype(np.uint32).view(np.float32).reshape(a.shape)


def _build():
    nc = bacc.Bacc(None, target_bir_lowering=False, debug=True)

    xt = nc.declare_dram_parameter("xt", [H, BL], F32R, isOutput=False)
    wq = nc.declare_dram_parameter("wq", [H, OPC], F32R, isOutput=False)
    wk = nc.declare_dram_parameter("wk", [H, OPC], F32R, isOutput=False)
    wv = nc.declare_dram_parameter("wv", [H, OPC], F32R, isOutput=False)
    wo = nc.declare_dram_parameter("wo", [OPC, H], F32R, isOutput=False)
    out = nc.declare_dram_parameter("out", [H, BL], F32, isOutput=True)

    with tile.TileContext(nc) as tc:
        with tc.tile_pool(name="persist", bufs=1) as persist:
            qt_sb = persist.tile([128, HPC, BL], F32R, tag="qt")
            kt_sb = persist.tile([128, HPC, BL], F32R, tag="kt")
            v_sb = persist.tile([128, BL // 128, OPC], F32R, tag="v")
            ones_sb = persist.tile([128, 128], F32R, tag="ones")

            # ---------------- phase 1: q/k/v projections ----------------
            with tc.tile_pool(name="wpool", bufs=1) as wpool, \
                 tc.tile_pool(name="xpool", bufs=2) as xpool, \
                 tc.tile_pool(name="ps1", bufs=6, space="PSUM") as ps1, \
                 tc.tile_pool(name="misc1", bufs=1) as misc1:
                ones_f = misc1.tile([128, 128], F32)
                nc.vector.memset(ones_f[:, :], 1.0)
                nc.vector.tensor_copy(ones_sb[:, :], ones_f[:, :])

                wq_sb = wpool.tile([128, HT, OPC], F32R, tag="wq")
                wk_sb = wpool.tile([128, HT, OPC], F32R, tag="wk")
                wv_sb = wpool.tile([128, HT, OPC], F32R, tag="wv")
                nc.sync.dma_start(
                    out=wq_sb[:, :, :],
                    in_=wq[:, :].rearrange("(t p) f -> p t f", p=128))
                nc.sync.dma_start(
                    out=wk_sb[:, :, :],
                    in_=wk[:, :].rearrange("(t p) f -> p t f", p=128))
                nc.sync.dma_start(
                    out=wv_sb[:, :, :],
                    in_=wv[:, :].rearrange("(t p) f -> p t f", p=128))

                for ic in range(N_IC1):
                    xch = xpool.tile([128, HT, IC1], F32R, tag="xch")
                    nc.sync.dma_start(
                        out=xch[:, :, :],
                        in_=xt[:, ic * IC1:(ic + 1) * IC1]
                        .rearrange("(t p) f -> p t f", p=128))
                    # q^T and k^T: (o_local x i), stationary = W^T h-tiles
                    for wsb, dest in ((wq_sb, qt_sb), (wk_sb, kt_sb)):
                        for ot in range(HPC):
                            ps = ps1.tile([128, IC1], F32, tag="ps")
                            for ht in range(HT):
                                nc.tensor.matmul(
                                    ps[:, :],
                                    wsb[:, ht, ot * 128:(ot + 1) * 128],
                                    xch[:, ht, :],
                                    start=(ht == 0), stop=(ht == HT - 1))
                            nc.scalar.copy(
                                dest[:, ot, ic * IC1:(ic + 1) * IC1], ps[:, :])
                    # v in natural (j x o) layout, stationary = x^T tiles
                    for it in range(IC1 // 128):
                        ps = ps1.tile([128, OPC], F32, tag="ps")
                        for ht in range(HT):
                            nc.tensor.matmul(
                                ps[:, :],
                                xch[:, ht, it * 128:(it + 1) * 128],
                                wv_sb[:, ht, :],
                                start=(ht == 0), stop=(ht == HT - 1))
                        nc.scalar.copy(
                            v_sb[:, ic * (IC1 // 128) + it, :], ps[:, :])

            # ---------------- phase 2: attention + o-projection ----------------
            with tc.tile_pool(name="wo_pool", bufs=1) as wo_pool, \
                 tc.tile_pool(name="exp_pool", bufs=4) as exp_pool, \
                 tc.tile_pool(name="acc_pool", bufs=2) as acc_pool, \
                 tc.tile_pool(name="sm_pool", bufs=2) as sm_pool, \
                 tc.tile_pool(name="mst_pool", bufs=2) as mst_pool, \
                 tc.tile_pool(name="oc_pool", bufs=4) as oc_pool, \
                 tc.tile_pool(name="scps", bufs=2, space="PSUM") as scps, \
                 tc.tile_pool(name="mxps", bufs=2, space="PSUM") as mxps, \
                 tc.tile_pool(name="rsps", bufs=2, space="PSUM") as rsps, \
                 tc.tile_pool(name="ops", bufs=2, space="PSUM") as ops:
                wo_sb = wo_pool.tile([128, HPC, H], F32R, tag="wo")
                nc.sync.dma_start(
                    out=wo_sb[:, :, :],
                    in_=wo[:, :].rearrange("(t p) f -> p t f", p=128))

                for b in range(B):
                    for ic in range(N_IC2):
                        gio = b * L + ic * IC2
                        njt = 4 * ic + 4      # causal: j-tiles 0..4ic+3
                        mst = mst_pool.tile([128, HPC, IC2], F32R, tag="mst")
                        for h in range(HPC):
                            mx = mxps.tile([128, IC2], F32, tag="mx")
                            acc = acc_pool.tile([128, IC2], F32, tag="acc")
                            nc.vector.memset(acc[:, :], 0.0)
                            for jt in range(njt):
                                sc = scps.tile([128, IC2], F32, tag="sc")
                                nc.tensor.matmul(
                                    sc[:, :],
                                    kt_sb[:, h, b * L + jt * 128:
                                          b * L + (jt + 1) * 128],
                                    qt_sb[:, h, gio:gio + IC2],
                                    start=True, stop=True)
                                ex = exp_pool.tile([128, IC2], F32R, tag="ex")
                                nc.scalar.activation(
                                    ex[:, :], sc[:, :],
                                    mybir.ActivationFunctionType.Exp,
                                    scale=SCALE)
                                if jt >= 4 * ic:
                                    # zero where j > i on diagonal tiles
                                    nc.gpsimd.affine_select(
                                        ex[:, :], ex[:, :],
                                        pattern=[[1, IC2]],
                                        compare_op=mybir.AluOpType.is_ge,
                                        fill=0.0,
                                        base=IC2 * ic - 128 * jt,
                                        channel_multiplier=-1)
                                nc.vector.tensor_add(
                                    acc[:, :], acc[:, :], ex[:, :])
                                nc.tensor.matmul(
                                    mx[:, :],
                                    v_sb[:, b * JT + jt,
                                         h * 128:(h + 1) * 128],
                                    ex[:, :],
                                    start=(jt == 0), stop=(jt == njt - 1))
                            accr = sm_pool.tile([128, IC2], F32R, tag="accr")
                            nc.vector.tensor_copy(accr[:, :], acc[:, :])
                            rs = rsps.tile([128, IC2], F32, tag="rs")
                            nc.tensor.matmul(rs[:, :], ones_sb[:, :],
                                             accr[:, :], start=True, stop=True)
                            rec = sm_pool.tile([128, IC2], F32, tag="rec")
                            nc.vector.reciprocal(rec[:, :], rs[:, :])
                            nc.vector.tensor_mul(mst[:, h, :], mx[:, :],
                                                 rec[:, :])
                        # partial o-projection for this (b, i-chunk)
                        for ot in range(H // 128):
                            op = ops.tile([128, IC2], F32, tag="op")
                            for hh in range(HPC):
                                nc.tensor.matmul(
                                    op[:, :],
                                    wo_sb[:, hh, ot * 128:(ot + 1) * 128],
                                    mst[:, hh, :],
                                    start=(hh == 0), stop=(hh == HPC - 1))
                            oc = oc_pool.tile([128, IC2], F32, tag="oc")
                            if ot % 2 == 0:
                                nc.scalar.copy(oc[:, :], op[:, :])
                            else:
                                nc.vector.tensor_copy(oc[:, :], op[:, :])
                            nc.sync.dma_start(
                                out=out[ot * 128:(ot + 1) * 128,
                                        gio:gio + IC2],
                                in_=oc[:, :])
    nc.finalize()
    return nc


_NC_CACHE = None


def _get_nc():
    global _NC_CACHE
    if _NC_CACHE is None:
        _NC_CACHE = _build()
    return _NC_CACHE


def _enable_profiling():
    """Wire the axon NTFF profile hook (missing antenv.axon_hooks shim)."""
    import types
    import antenv
    if "antenv.axon_hooks" not in sys.modules:
        shim = types.ModuleType("antenv.axon_hooks")

        def set_axon_ntff_profile_hook(h):
            shim._the_hook = h

        def get_axon_ntff_profile_hook():
            return getattr(shim, "_the_hook", None)

        shim.set_axon_ntff_profile_hook = set_axon_ntff_profile_hook
        shim.get_axon_ntff_profile_hook = get_axon_ntff_profile_hook
        sys.modules["antenv.axon_hooks"] = shim
        antenv.axon_hooks = shim
    from trn_agent_boot.trn_boot import _ntff_profile_via_ctypes
    hook = _ntff_profile_via_ctypes("/opt/axon/libaxon_pjrt.so")
    sys.modules["antenv.axon_hooks"].set_axon_ntff_profile_hook(hook)
    import concourse.bass_utils as bu
    bu.upload_artifacts = lambda tmpdir: "local://" + tmpdir


def kernel(x, padding_mask, Wq, Wk, Wv, Wo):
    global LAST_EXEC_NS
    x = np.asarray(x, dtype=np.float32)
    Wq = np.asarray(Wq, dtype=np.float32)
    Wk = np.asarray(Wk, dtype=np.float32)
    Wv = np.asarray(Wv, dtype=np.float32)
    Wo = np.asarray(Wo, dtype=np.float32)

    xt = _round_fp32r(x.reshape(BL, H).T)        # (H, BL)
    wqt = _round_fp32r(Wq.T)                     # (H, H): [h, o]
    wkt = _round_fp32r(Wk.T)
    wvt = _round_fp32r(Wv.T)
    wot = _round_fp32r(Wo.T)                     # (H, H): [h_in, o]

    in_maps = []
    for c in range(N_CORES):
        sl = slice(c * OPC, (c + 1) * OPC)
        in_maps.append({
            "xt": xt,
            "wq": np.ascontiguousarray(wqt[:, sl]),
            "wk": np.ascontiguousarray(wkt[:, sl]),
            "wv": np.ascontiguousarray(wvt[:, sl]),
            "wo": np.ascontiguousarray(wot[sl, :]),
        })

    profile = os.environ.get("KERNEL_PROFILE", "0") == "1"
    if profile:
        _enable_profiling()

    nc = _get_nc()
    res = run_bass_kernel_spmd(nc, in_maps, core_ids=list(range(N_CORES)),
                               trace=profile)
    LAST_EXEC_NS = res.exec_time_ns

    total = np.zeros((H, BL), dtype=np.float64)
    for c in range(N_CORES):
        total += res.results[c]["out"]
    return np.ascontiguousarray(total.T).astype(np.float32).reshape(B, L, H)


# revision 3
# speedup vs baseline: 1.0811x; 1.0811x over previous
"""Multi-head causal attention (B=2, L=2048, H=2048, NH=16) on 8 Trainium2
NeuronCores.

Sharding: tensor-parallel over heads — core c computes heads {2c, 2c+1}.
Each core:
  phase 1: q/k/v projections for its 256 output dims (contract over H=2048)
  phase 2: causal attention for its 2 heads + its partial o-projection
Host: transposes/rounds inputs (fp32r layout prep), sums the 8 partial
o-projection outputs, and transposes back.

All matmuls run in float32r (fp32 with 11-bit mantissa, 1 cycle/row on the
PE for free dims >= 256 — 4x faster than plain fp32 at ~2.4e-4 rounding).

Phase-2 softmax is structured to keep the PE dense (HAM stays warm):
  - colsum of exp accumulates on the PE via a ones-matmul per j-tile
    (PSUM accumulation), not a DVE add chain
  - reciprocal runs on a single (1 x 512) row, then gpsimd
    partition_broadcast replicates it
  - o-projection of chunk N is emitted after attention of chunk N+1 so the
    PE never waits for the softmax normalize chain
  - causally-masked j-tiles are skipped; diagonal j-tiles stream only the
    live i-columns (floor 256 — below that fp32r drops to 4 cyc/row)
"""

import os
import sys

if "/opt/trn_rl_repo" not in sys.path:
    sys.path.insert(0, "/opt/trn_rl_repo")

import numpy as np

from concourse import bacc, mybir, tile  # noqa: E402
from concourse.bass_utils import run_bass_kernel_spmd  # noqa: E402

F32R = mybir.dt.float32r
F32 = mybir.dt.float32

N_CORES = 8
B, L, H, NH = 2, 2048, 2048, 16
DH = H // NH                       # 128
BL = B * L                        # 4096
HPC = NH // N_CORES               # heads per core = 2
OPC = HPC * DH                    # output dims per core = 256
HT = H // 128                     # 16 h-tiles (contraction)
IC1 = 256                         # phase-1 i-chunk width
N_IC1 = BL // IC1                 # 16
IC2 = 512                         # phase-2 i-chunk width
N_IC2 = L // IC2                  # 4 per batch
JT = L // 128                     # 16 j-tiles per batch
SCALE = 1.0 / float(np.sqrt(DH))

LAST_EXEC_NS = None


def _round_fp32r(a: np.ndarray) -> np.ndarray:
    """Round fp32 to fp32r (11-bit mantissa, round-to-nearest-even)."""
    a = np.ascontiguousarray(a, dtype=np.float32)
    u = a.view(np.uint32)
    low = u & np.uint32(0xFFF)
    rounded = (u & np.uint32(0xFFFFF000)).astype(np.uint64)
    half = np.uint32(0x800)
    lsb = (u >> np.uint32(12)) & np.uint32(1)
    up = (low > half) | ((low == half) & (lsb == 1))
    rounded = rounded + (up.astype(np.uint64) << np.uint64(12))
    return rounded.astype(np.uint32).view(np.float32).reshape(a.shape)


def _build():
    nc = bacc.Bacc(None, target_bir_lowering=False, debug=True)

    xt = nc.declare_dram_parameter("xt", [H, BL], F32R, isOutput=False)
    wq = nc.declare_dram_parameter("wq", [H, OPC], F32R, isOutput=False)
    wk = nc.declare_dram_parameter("wk", [H, OPC], F32R, isOutput=False)
    wv = nc.declare_dram_parameter("wv", [H, OPC], F32R, isOutput=False)
    wo = nc.declare_dram_parameter("wo", [OPC, H], F32R, isOutput=False)
    out = nc.declare_dram_parameter("out", [H, BL], F32, isOutput=True)

    with tile.TileContext(nc) as tc:
        with tc.tile_pool(name="persist", bufs=1) as persist:
            qt_sb = persist.tile([128, HPC, BL], F32R, tag="qt")
            kt_sb = persist.tile([128, HPC, BL], F32R, tag="kt")
            v_sb = persist.tile([128, BL // 128, OPC], F32R, tag="v")
            ones_sb = persist.tile([128, 128], F32R, tag="ones")

            # ---------------- phase 1: q/k/v projections ----------------
            with tc.tile_pool(name="wpool", bufs=1) as wpool, \
                 tc.tile_pool(name="xpool", bufs=2) as xpool, \
                 tc.tile_pool(name="ps1", bufs=6, space="PSUM") as ps1, \
                 tc.tile_pool(name="misc1", bufs=1) as misc1:
                wq_sb = wpool.tile([128, HT, OPC], F32R, tag="wq")
                wk_sb = wpool.tile([128, HT, OPC], F32R, tag="wk")
                wv_sb = wpool.tile([128, HT, OPC], F32R, tag="wv")
                # wq on the scalar HWDGE queue, first x-chunk on the sync
                # queue — they land in parallel so the PE starts early.
                nc.scalar.dma_start(
                    out=wq_sb[:, :, :],
                    in_=wq[:, :].rearrange("(t p) f -> p t f", p=128))

                xchs = {}
                xchs[0] = xpool.tile([128, HT, IC1], F32R, tag="xch",
                                     name="xch")
                nc.sync.dma_start(
                    out=xchs[0][:, :, :],
                    in_=xt[:, 0:IC1].rearrange("(t p) f -> p t f", p=128))

                nc.scalar.dma_start(
                    out=wk_sb[:, :, :],
                    in_=wk[:, :].rearrange("(t p) f -> p t f", p=128))
                nc.scalar.dma_start(
                    out=wv_sb[:, :, :],
                    in_=wv[:, :].rearrange("(t p) f -> p t f", p=128))

                ones_f = misc1.tile([128, 128], F32)
                nc.vector.memset(ones_f[:, :], 1.0)
                nc.vector.tensor_copy(ones_sb[:, :], ones_f[:, :])

                for ic in range(N_IC1):
                    if ic not in xchs:
                        xchs[ic] = xpool.tile([128, HT, IC1], F32R,
                                              tag="xch", name="xch")
                        nc.sync.dma_start(
                            out=xchs[ic][:, :, :],
                            in_=xt[:, ic * IC1:(ic + 1) * IC1]
                            .rearrange("(t p) f -> p t f", p=128))
                    xch = xchs.pop(ic)
                    # q^T and k^T: (o_local x i), stationary = W^T h-tiles
                    ncopy = 0
                    for wsb, dest in ((wq_sb, qt_sb), (wk_sb, kt_sb)):
                        for ot in range(HPC):
                            ps = ps1.tile([128, IC1], F32, tag="ps")
                            for ht in range(HT):
                                nc.tensor.matmul(
                                    ps[:, :],
                                    wsb[:, ht, ot * 128:(ot + 1) * 128],
                                    xch[:, ht, :],
                                    start=(ht == 0), stop=(ht == HT - 1))
                            if ncopy % 2 == 0:
                                nc.scalar.copy(
                                    dest[:, ot, ic * IC1:(ic + 1) * IC1],
                                    ps[:, :])
                            else:
                                nc.vector.tensor_copy(
                                    dest[:, ot, ic * IC1:(ic + 1) * IC1],
                                    ps[:, :])
                            ncopy += 1
                    # v in natural (j x o) layout, stationary = x^T tiles
                    for it in range(IC1 // 128):
                        ps = ps1.tile([128, OPC], F32, tag="ps")
                        for ht in range(HT):
                            nc.tensor.matmul(
                                ps[:, :],
                                xch[:, ht, it * 128:(it + 1) * 128],
                                wv_sb[:, ht, :],
                                start=(ht == 0), stop=(ht == HT - 1))
                        if it % 2 == 0:
                            nc.scalar.copy(
                                v_sb[:, ic * (IC1 // 128) + it, :], ps[:, :])
                        else:
                            nc.vector.tensor_copy(
                                v_sb[:, ic * (IC1 // 128) + it, :], ps[:, :])

            # ---------- phase 2: attention + pipelined o-projection ----------
            with tc.tile_pool(name="wo_pool", bufs=1) as wo_pool, \
                 tc.tile_pool(name="exp_pool", bufs=4) as exp_pool, \
                 tc.tile_pool(name="sm_pool", bufs=2) as sm_pool, \
                 tc.tile_pool(name="mst_pool", bufs=2) as mst_pool, \
                 tc.tile_pool(name="oc_pool", bufs=4) as oc_pool, \
                 tc.tile_pool(name="scps", bufs=2, space="PSUM") as scps, \
                 tc.tile_pool(name="mxps", bufs=2, space="PSUM") as mxps, \
                 tc.tile_pool(name="rsps", bufs=2, space="PSUM") as rsps, \
                 tc.tile_pool(name="ops", bufs=2, space="PSUM") as ops:
                wo_sb = wo_pool.tile([128, HPC, H], F32R, tag="wo")
                nc.scalar.dma_start(
                    out=wo_sb[:, :, :],
                    in_=wo[:, :].rearrange("(t p) f -> p t f", p=128))

                def emit_oproj(mst, gio):
                    for ot in range(H // 128):
                        op = ops.tile([128, IC2], F32, tag="op")
                        for hh in range(HPC):
                            nc.tensor.matmul(
                                op[:, :],
                                wo_sb[:, hh, ot * 128:(ot + 1) * 128],
                                mst[:, hh, :],
                                start=(hh == 0), stop=(hh == HPC - 1))
                        oc = oc_pool.tile([128, IC2], F32, tag="oc")
                        if ot % 2 == 0:
                            nc.scalar.copy(oc[:, :], op[:, :])
                        else:
                            nc.vector.tensor_copy(oc[:, :], op[:, :])
                        nc.sync.dma_start(
                            out=out[ot * 128:(ot + 1) * 128, gio:gio + IC2],
                            in_=oc[:, :])

                pending = None
                for b in range(B):
                    for ic in range(N_IC2):
                        gio = b * L + ic * IC2
                        njt = 4 * ic + 4      # causal: j-tiles 0..4ic+3
                        mst = mst_pool.tile([128, HPC, IC2], F32R, tag="mst")
                        for h in range(HPC):
                            mx = mxps.tile([128, IC2], F32, tag="mx")
                            rs = rsps.tile([128, IC2], F32, tag="rs")
                            for jt in range(njt):
                                # live i-columns: i >= j on diagonal tiles;
                                # keep width >= 256 for fp32r full rate
                                f0 = min(max(0, 128 * jt - IC2 * ic), IC2 - 256)
                                w = IC2 - f0
                                sc = scps.tile([128, IC2], F32, tag="sc")
                                nc.tensor.matmul(
                                    sc[:, f0:],
                                    kt_sb[:, h, b * L + jt * 128:
                                          b * L + (jt + 1) * 128],
                                    qt_sb[:, h, gio + f0:gio + IC2],
                                    start=True, stop=True)
                                ex = exp_pool.tile([128, IC2], F32R, tag="ex")
                                nc.scalar.activation(
                                    ex[:, f0:], sc[:, f0:],
                                    mybir.ActivationFunctionType.Exp,
                                    scale=SCALE)
                                if jt >= 4 * ic:
                                    # zero where j > i
                                    nc.gpsimd.affine_select(
                                        ex[:, f0:], ex[:, f0:],
                                        pattern=[[1, w]],
                                        compare_op=mybir.AluOpType.is_ge,
                                        fill=0.0,
                                        base=f0 - (128 * jt - IC2 * ic),
                                        channel_multiplier=-1)
                                nc.tensor.matmul(
                                    rs[:, f0:], ones_sb[:, :], ex[:, f0:],
                                    start=(jt == 0), stop=(jt == njt - 1))
                                nc.tensor.matmul(
                                    mx[:, f0:],
                                    v_sb[:, b * JT + jt,
                                         h * 128:(h + 1) * 128],
                                    ex[:, f0:],
                                    start=(jt == 0), stop=(jt == njt - 1))
                            rec_row = sm_pool.tile([1, IC2], F32, tag="recrow")
                            nc.vector.reciprocal(rec_row[:, :], rs[0:1, :])
                            rec_sb = sm_pool.tile([128, IC2], F32, tag="recb")
                            nc.gpsimd.partition_broadcast(
                                rec_sb[:, :], rec_row[:, :], channels=128)
                            nc.vector.tensor_mul(mst[:, h, :], mx[:, :],
                                                 rec_sb[:, :])
                        if pending is not None:
                            emit_oproj(*pending)
                        pending = (mst, gio)
                emit_oproj(*pending)
    nc.finalize()
    return nc


_NC_CACHE = None


def _get_nc():
    global _NC_CACHE
    if _NC_CACHE is None:
        _NC_CACHE = _build()
    return _NC_CACHE


def _enable_profiling():
    """Wire the axon NTFF profile hook (missing antenv.axon_hooks shim)."""
    import types
    import antenv
    if "antenv.axon_hooks" not in sys.modules:
        shim = types.ModuleType("antenv.axon_hooks")

        def set_axon_ntff_profile_hook(h):
            shim._the_hook = h

        def get_axon_ntff_profile_hook():
            return getattr(shim, "_the_hook", None)

        shim.set_axon_ntff_profile_hook = set_axon_ntff_profile_hook
        shim.get_axon_ntff_profile_hook = get_axon_ntff_profile_hook
        sys.modules["antenv.axon_hooks"] = shim
        antenv.axon_hooks = shim
    from trn_agent_boot.trn_boot import _ntff_profile_via_ctypes
    hook = _ntff_profile_via_ctypes("/opt/axon/libaxon_pjrt.so")
    sys.modules["antenv.axon_hooks"].set_axon_ntff_profile_hook(hook)
    import concourse.bass_utils as bu
    bu.upload_artifacts = lambda tmpdir: "local://" + tmpdir


def kernel(x, padding_mask, Wq, Wk, Wv, Wo):
    global LAST_EXEC_NS
    x = np.asarray(x, dtype=np.float32)
    Wq = np.asarray(Wq, dtype=np.float32)
    Wk = np.asarray(Wk, dtype=np.float32)
    Wv = np.asarray(Wv, dtype=np.float32)
    Wo = np.asarray(Wo, dtype=np.float32)

    xt = _round_fp32r(x.reshape(BL, H).T)        # (H, BL)
    wqt = _round_fp32r(Wq.T)                     # (H, H): [h, o]
    wkt = _round_fp32r(Wk.T)
    wvt = _round_fp32r(Wv.T)
    wot = _round_fp32r(Wo.T)                     # (H, H): [h_in, o]

    in_maps = []
    for c in range(N_CORES):
        sl = slice(c * OPC, (c + 1) * OPC)
        in_maps.append({
            "xt": xt,
            "wq": np.ascontiguousarray(wqt[:, sl]),
            "wk": np.ascontiguousarray(wkt[:, sl]),
            "wv": np.ascontiguousarray(wvt[:, sl]),
            "wo": np.ascontiguousarray(wot[sl, :]),
        })

    profile = os.environ.get("KERNEL_PROFILE", "0") == "1"
    if profile:
        _enable_profiling()

    nc = _get_nc()
    res = run_bass_kernel_spmd(nc, in_maps, core_ids=list(range(N_CORES)),
                               trace=profile)
    LAST_EXEC_NS = res.exec_time_ns

    total = np.zeros((H, BL), dtype=np.float64)
    for c in range(N_CORES):
        total += res.results[c]["out"]
    return np.ascontiguousarray(total.T).astype(np.float32).reshape(B, L, H)


# revision 4
# speedup vs baseline: 1.1249x; 1.0406x over previous
"""Multi-head causal attention (B=2, L=2048, H=2048, NH=16) on 8 Trainium2
NeuronCores.

Sharding: tensor-parallel over heads — core c computes heads {2c, 2c+1}.
Each core:
  phase 1: q/k/v projections for its 256 output dims (contract over H=2048)
  phase 2: causal attention for its 2 heads + its partial o-projection
Host: transposes/rounds inputs (fp32r layout prep), sums the 8 partial
o-projection outputs, and transposes back.

All matmuls run in float32r (fp32 with 11-bit mantissa, 1 cycle/row on the
PE for free dims >= 256 — 4x faster than plain fp32 at ~2.4e-4 rounding).

Phase-2 softmax is structured to keep the PE dense (HAM stays warm):
  - colsum of exp accumulates on the PE via a ones-matmul per j-tile
    (PSUM accumulation), not a DVE add chain
  - reciprocal runs on a single (1 x 512) row, then gpsimd
    partition_broadcast replicates it
  - o-projection of chunk N is emitted after attention of chunk N+1 so the
    PE never waits for the softmax normalize chain
  - causally-masked j-tiles are skipped; diagonal j-tiles stream only the
    live i-columns (floor 256 — below that fp32r drops to 4 cyc/row)
"""

import os
import sys

if "/opt/trn_rl_repo" not in sys.path:
    sys.path.insert(0, "/opt/trn_rl_repo")

import numpy as np

from concourse import bacc, mybir, tile  # noqa: E402
from concourse.bass_utils import run_bass_kernel_spmd  # noqa: E402

F32R = mybir.dt.float32r
F32 = mybir.dt.float32

N_CORES = 8
B, L, H, NH = 2, 2048, 2048, 16
DH = H // NH                       # 128
BL = B * L                        # 4096
HPC = NH // N_CORES               # heads per core = 2
OPC = HPC * DH                    # output dims per core = 256
HT = H // 128                     # 16 h-tiles (contraction)
IC1 = 256                         # phase-1 i-chunk width
N_IC1 = BL // IC1                 # 16
IC2 = 512                         # phase-2 i-chunk width
N_IC2 = L // IC2                  # 4 per batch
JT = L // 128                     # 16 j-tiles per batch
SCALE = 1.0 / float(np.sqrt(DH))

LAST_EXEC_NS = None


def _round_fp32r(a: np.ndarray) -> np.ndarray:
    """Round fp32 to fp32r (11-bit mantissa, round-to-nearest-even)."""
    a = np.ascontiguousarray(a, dtype=np.float32)
    u = a.view(np.uint32)
    low = u & np.uint32(0xFFF)
    rounded = (u & np.uint32(0xFFFFF000)).astype(np.uint64)
    half = np.uint32(0x800)
    lsb = (u >> np.uint32(12)) & np.uint32(1)
    up = (low > half) | ((low == half) & (lsb == 1))
    rounded = rounded + (up.astype(np.uint64) << np.uint64(12))
    return rounded.astype(np.uint32).view(np.float32).reshape(a.shape)


def _build():
    nc = bacc.Bacc(None, target_bir_lowering=False, debug=True)

    xt = nc.declare_dram_parameter("xt", [H, BL], F32R, isOutput=False)
    wq = nc.declare_dram_parameter("wq", [H, OPC], F32R, isOutput=False)
    wk = nc.declare_dram_parameter("wk", [H, OPC], F32R, isOutput=False)
    wv = nc.declare_dram_parameter("wv", [H, OPC], F32R, isOutput=False)
    wo = nc.declare_dram_parameter("wo", [OPC, H], F32R, isOutput=False)
    out = nc.declare_dram_parameter("out", [H, BL], F32, isOutput=True)

    with tile.TileContext(nc) as tc:
        with tc.tile_pool(name="persist", bufs=1) as persist:
            qt_sb = persist.tile([128, HPC, BL], F32R, tag="qt")
            kt_sb = persist.tile([128, HPC, BL], F32R, tag="kt")
            v_sb = persist.tile([128, BL // 128, OPC], F32R, tag="v")
            ones_sb = persist.tile([128, 128], F32R, tag="ones")

            # ---------------- phase 1: q/k/v projections ----------------
            with tc.tile_pool(name="wpool", bufs=1) as wpool, \
                 tc.tile_pool(name="xpool", bufs=2) as xpool, \
                 tc.tile_pool(name="ps1", bufs=4, space="PSUM") as ps1, \
                 tc.tile_pool(name="misc1", bufs=1) as misc1:
                wq_sb = wpool.tile([128, HT, OPC], F32R, tag="wq")
                wk_sb = wpool.tile([128, HT, OPC], F32R, tag="wk")
                wv_sb = wpool.tile([128, HT, OPC], F32R, tag="wv")
                # Fine-grained startup DMAs alternating across the two
                # HWDGE queues so the first matmul's inputs land fast.
                xchs = {}
                xchs[0] = xpool.tile([128, HT, IC1], F32R, tag="xch",
                                     name="xch")
                for ht in range(HT):
                    eng = nc.sync if ht % 2 == 0 else nc.scalar
                    eng.dma_start(
                        out=wq_sb[:, ht, :],
                        in_=wq[ht * 128:(ht + 1) * 128, :])
                    eng = nc.scalar if ht % 2 == 0 else nc.sync
                    eng.dma_start(
                        out=xchs[0][:, ht, :],
                        in_=xt[ht * 128:(ht + 1) * 128, 0:IC1])
                for i in range(4):
                    qt4 = HT // 4
                    nc.sync.dma_start(
                        out=wk_sb[:, i * qt4:(i + 1) * qt4, :],
                        in_=wk[:, :].rearrange("(q t p) f -> q p t f",
                                               q=4, p=128)[i])
                    nc.scalar.dma_start(
                        out=wv_sb[:, i * qt4:(i + 1) * qt4, :],
                        in_=wv[:, :].rearrange("(q t p) f -> q p t f",
                                               q=4, p=128)[i])

                ones_f = misc1.tile([128, 128], F32)
                nc.vector.memset(ones_f[:, :], 1.0)
                nc.vector.tensor_copy(ones_sb[:, :], ones_f[:, :])

                for ic in range(N_IC1):
                    if ic not in xchs:
                        xchs[ic] = xpool.tile([128, HT, IC1], F32R,
                                              tag="xch", name="xch")
                        eng = nc.sync if ic % 2 == 0 else nc.scalar
                        eng.dma_start(
                            out=xchs[ic][:, :, :],
                            in_=xt[:, ic * IC1:(ic + 1) * IC1]
                            .rearrange("(t p) f -> p t f", p=128))
                    xch = xchs.pop(ic)
                    # q^T and k^T: (o_local x i), stationary = W^T h-tiles
                    ncopy = 0
                    for wsb, dest in ((wq_sb, qt_sb), (wk_sb, kt_sb)):
                        for ot in range(HPC):
                            ps = ps1.tile([128, IC1], F32, tag="ps")
                            for ht in range(HT):
                                nc.tensor.matmul(
                                    ps[:, :],
                                    wsb[:, ht, ot * 128:(ot + 1) * 128],
                                    xch[:, ht, :],
                                    start=(ht == 0), stop=(ht == HT - 1))
                            if ncopy % 2 == 0:
                                nc.scalar.copy(
                                    dest[:, ot, ic * IC1:(ic + 1) * IC1],
                                    ps[:, :])
                            else:
                                nc.vector.tensor_copy(
                                    dest[:, ot, ic * IC1:(ic + 1) * IC1],
                                    ps[:, :])
                            ncopy += 1
                    # v in natural (j x o) layout, stationary = x^T tiles
                    for it in range(IC1 // 128):
                        ps = ps1.tile([128, OPC], F32, tag="ps")
                        for ht in range(HT):
                            nc.tensor.matmul(
                                ps[:, :],
                                xch[:, ht, it * 128:(it + 1) * 128],
                                wv_sb[:, ht, :],
                                start=(ht == 0), stop=(ht == HT - 1))
                        if it % 2 == 0:
                            nc.scalar.copy(
                                v_sb[:, ic * (IC1 // 128) + it, :], ps[:, :])
                        else:
                            nc.vector.tensor_copy(
                                v_sb[:, ic * (IC1 // 128) + it, :], ps[:, :])

            # ---------- phase 2: attention + pipelined o-projection ----------
            with tc.tile_pool(name="wo_pool", bufs=1) as wo_pool, \
                 tc.tile_pool(name="exp_pool", bufs=4) as exp_pool, \
                 tc.tile_pool(name="sm_pool", bufs=2) as sm_pool, \
                 tc.tile_pool(name="mst_pool", bufs=2) as mst_pool, \
                 tc.tile_pool(name="oc_pool", bufs=4) as oc_pool, \
                 tc.tile_pool(name="scps", bufs=2, space="PSUM") as scps, \
                 tc.tile_pool(name="mxps", bufs=2, space="PSUM") as mxps, \
                 tc.tile_pool(name="rsps", bufs=2, space="PSUM") as rsps, \
                 tc.tile_pool(name="ops", bufs=2, space="PSUM") as ops:
                wo_sb = wo_pool.tile([128, HPC, H], F32R, tag="wo")
                nc.scalar.dma_start(
                    out=wo_sb[:, :, :],
                    in_=wo[:, :].rearrange("(t p) f -> p t f", p=128))

                def emit_oproj(mst, gio):
                    for ot in range(H // 128):
                        op = ops.tile([128, IC2], F32, tag="op")
                        for hh in range(HPC):
                            nc.tensor.matmul(
                                op[:, :],
                                wo_sb[:, hh, ot * 128:(ot + 1) * 128],
                                mst[:, hh, :],
                                start=(hh == 0), stop=(hh == HPC - 1))
                        oc = oc_pool.tile([128, IC2], F32, tag="oc")
                        nc.vector.tensor_copy(oc[:, :], op[:, :])
                        eng = nc.sync if ot % 2 == 0 else nc.scalar
                        eng.dma_start(
                            out=out[ot * 128:(ot + 1) * 128, gio:gio + IC2],
                            in_=oc[:, :])

                pending = None
                for b in range(B):
                    for ic in range(N_IC2):
                        gio = b * L + ic * IC2
                        njt = 4 * ic + 4      # causal: j-tiles 0..4ic+3
                        mst = mst_pool.tile([128, HPC, IC2], F32R, tag="mst")
                        for h in range(HPC):
                            mx = mxps.tile([128, IC2], F32, tag="mx")
                            rs = rsps.tile([1, IC2], F32, tag="rs")
                            for jt in range(njt):
                                # live i-columns: i >= j on diagonal tiles;
                                # keep width >= 256 for fp32r full rate
                                f0 = min(max(0, 128 * jt - IC2 * ic), IC2 - 256)
                                w = IC2 - f0
                                sc = scps.tile([128, IC2], F32, tag="sc")
                                nc.tensor.matmul(
                                    sc[:, f0:],
                                    kt_sb[:, h, b * L + jt * 128:
                                          b * L + (jt + 1) * 128],
                                    qt_sb[:, h, gio + f0:gio + IC2],
                                    start=True, stop=True)
                                ex = exp_pool.tile([128, IC2], F32R, tag="ex")
                                nc.scalar.activation(
                                    ex[:, f0:], sc[:, f0:],
                                    mybir.ActivationFunctionType.Exp,
                                    scale=SCALE)
                                if jt >= 4 * ic:
                                    # zero where j > i
                                    nc.gpsimd.affine_select(
                                        ex[:, f0:], ex[:, f0:],
                                        pattern=[[1, w]],
                                        compare_op=mybir.AluOpType.is_ge,
                                        fill=0.0,
                                        base=f0 - (128 * jt - IC2 * ic),
                                        channel_multiplier=-1)
                                nc.tensor.matmul(
                                    rs[:, f0:], ones_sb[:, 0:1], ex[:, f0:],
                                    start=(jt == 0), stop=(jt == njt - 1))
                                nc.tensor.matmul(
                                    mx[:, f0:],
                                    v_sb[:, b * JT + jt,
                                         h * 128:(h + 1) * 128],
                                    ex[:, f0:],
                                    start=(jt == 0), stop=(jt == njt - 1))
                            rec_row = sm_pool.tile([1, IC2], F32, tag="recrow")
                            nc.vector.reciprocal_approx_fast(
                                out=rec_row[:, :], in_=rs[0:1, :])
                            rec_sb = sm_pool.tile([128, IC2], F32, tag="recb")
                            nc.gpsimd.partition_broadcast(
                                rec_sb[:, :], rec_row[:, :], channels=128)
                            nc.vector.tensor_mul(mst[:, h, :], mx[:, :],
                                                 rec_sb[:, :])
                        if pending is not None:
                            emit_oproj(*pending)
                        pending = (mst, gio)
                emit_oproj(*pending)
    nc.finalize()
    return nc


_NC_CACHE = None


def _get_nc():
    global _NC_CACHE
    if _NC_CACHE is None:
        _NC_CACHE = _build()
    return _NC_CACHE


def _enable_profiling():
    """Wire the axon NTFF profile hook (missing antenv.axon_hooks shim)."""
    import types
    import antenv
    if "antenv.axon_hooks" not in sys.modules:
        shim = types.ModuleType("antenv.axon_hooks")

        def set_axon_ntff_profile_hook(h):
            shim._the_hook = h

        def get_axon_ntff_profile_hook():
            return getattr(shim, "_the_hook", None)

        shim.set_axon_ntff_profile_hook = set_axon_ntff_profile_hook
        shim.get_axon_ntff_profile_hook = get_axon_ntff_profile_hook
        sys.modules["antenv.axon_hooks"] = shim
        antenv.axon_hooks = shim
    from trn_agent_boot.trn_boot import _ntff_profile_via_ctypes
    hook = _ntff_profile_via_ctypes("/opt/axon/libaxon_pjrt.so")
    sys.modules["antenv.axon_hooks"].set_axon_ntff_profile_hook(hook)
    import concourse.bass_utils as bu
    bu.upload_artifacts = lambda tmpdir: "local://" + tmpdir


def kernel(x, padding_mask, Wq, Wk, Wv, Wo):
    global LAST_EXEC_NS
    x = np.asarray(x, dtype=np.float32)
    Wq = np.asarray(Wq, dtype=np.float32)
    Wk = np.asarray(Wk, dtype=np.float32)
    Wv = np.asarray(Wv, dtype=np.float32)
    Wo = np.asarray(Wo, dtype=np.float32)

    xt = _round_fp32r(x.reshape(BL, H).T)        # (H, BL)
    wqt = _round_fp32r(Wq.T)                     # (H, H): [h, o]
    wkt = _round_fp32r(Wk.T)
    wvt = _round_fp32r(Wv.T)
    wot = _round_fp32r(Wo.T)                     # (H, H): [h_in, o]

    in_maps = []
    for c in range(N_CORES):
        sl = slice(c * OPC, (c + 1) * OPC)
        in_maps.append({
            "xt": xt,
            "wq": np.ascontiguousarray(wqt[:, sl]),
            "wk": np.ascontiguousarray(wkt[:, sl]),
            "wv": np.ascontiguousarray(wvt[:, sl]),
            "wo": np.ascontiguousarray(wot[sl, :]),
        })

    profile = os.environ.get("KERNEL_PROFILE", "0") == "1"
    if profile:
        _enable_profiling()

    nc = _get_nc()
    res = run_bass_kernel_spmd(nc, in_maps, core_ids=list(range(N_CORES)),
                               trace=profile)
    LAST_EXEC_NS = res.exec_time_ns

    total = np.zeros((H, BL), dtype=np.float64)
    for c in range(N_CORES):
        total += res.results[c]["out"]
    return np.ascontiguousarray(total.T).astype(np.float32).reshape(B, L, H)


# revision 5
# speedup vs baseline: 1.1569x; 1.0285x over previous
"""Multi-head causal attention (B=2, L=2048, H=2048, NH=16) on 8 Trainium2
NeuronCores.

Sharding: tensor-parallel over heads — core c computes heads {2c, 2c+1}.
Each core:
  phase 1: q/k/v projections for its 256 output dims (contract over H=2048)
  phase 2: causal attention for its 2 heads + its partial o-projection
Host: transposes/rounds inputs (fp32r layout prep), sums the 8 partial
o-projection outputs, and transposes back.

All matmuls run in float32r (fp32 with 11-bit mantissa, 1 cycle/row on the
PE for free dims >= 256 — 4x faster than plain fp32 at ~2.4e-4 rounding).

Phase-2 softmax is structured to keep the PE dense (HAM stays warm):
  - colsum of exp accumulates on the PE via a ones-matmul per j-tile
    (PSUM accumulation), not a DVE add chain
  - reciprocal runs on a single (1 x 512) row, then gpsimd
    partition_broadcast replicates it
  - o-projection of chunk N is emitted after attention of chunk N+1 so the
    PE never waits for the softmax normalize chain
  - causally-masked j-tiles are skipped; diagonal j-tiles stream only the
    live i-columns (floor 256 — below that fp32r drops to 4 cyc/row)
"""

import os
import sys

if "/opt/trn_rl_repo" not in sys.path:
    sys.path.insert(0, "/opt/trn_rl_repo")

import numpy as np

from concourse import bacc, mybir, tile  # noqa: E402
from concourse.bass_utils import run_bass_kernel_spmd  # noqa: E402

F32R = mybir.dt.float32r
F32 = mybir.dt.float32

N_CORES = 8
B, L, H, NH = 2, 2048, 2048, 16
DH = H // NH                       # 128
BL = B * L                        # 4096
HPC = NH // N_CORES               # heads per core = 2
OPC = HPC * DH                    # output dims per core = 256
HT = H // 128                     # 16 h-tiles (contraction)
IC1 = 256                         # phase-1 i-chunk width
N_IC1 = BL // IC1                 # 16
IC2 = 512                         # phase-2 i-chunk width
N_IC2 = L // IC2                  # 4 per batch
JT = L // 128                     # 16 j-tiles per batch
SCALE = 1.0 / float(np.sqrt(DH))

LAST_EXEC_NS = None


def _round_fp32r(a: np.ndarray) -> np.ndarray:
    """Round fp32 to fp32r (11-bit mantissa, round-to-nearest-even)."""
    a = np.ascontiguousarray(a, dtype=np.float32)
    u = a.view(np.uint32)
    low = u & np.uint32(0xFFF)
    rounded = (u & np.uint32(0xFFFFF000)).astype(np.uint64)
    half = np.uint32(0x800)
    lsb = (u >> np.uint32(12)) & np.uint32(1)
    up = (low > half) | ((low == half) & (lsb == 1))
    rounded = rounded + (up.astype(np.uint64) << np.uint64(12))
    return rounded.astype(np.uint32).view(np.float32).reshape(a.shape)


def _build():
    nc = bacc.Bacc(None, target_bir_lowering=False, debug=True)

    xt = nc.declare_dram_parameter("xt", [H, BL], F32R, isOutput=False)
    wq = nc.declare_dram_parameter("wq", [H, OPC], F32R, isOutput=False)
    wk = nc.declare_dram_parameter("wk", [H, OPC], F32R, isOutput=False)
    wv = nc.declare_dram_parameter("wv", [H, OPC], F32R, isOutput=False)
    wo = nc.declare_dram_parameter("wo", [OPC, H], F32R, isOutput=False)
    out = nc.declare_dram_parameter("out", [H, BL], F32, isOutput=True)

    with tile.TileContext(nc) as tc:
        with tc.tile_pool(name="persist", bufs=1) as persist:
            qt_sb = persist.tile([128, HPC, BL], F32R, tag="qt")
            kt_sb = persist.tile([128, HPC, BL], F32R, tag="kt")
            v_sb = persist.tile([128, BL // 128, OPC], F32R, tag="v")
            ones_sb = persist.tile([128, 128], F32R, tag="ones")

            # ---------------- phase 1: q/k/v projections ----------------
            with tc.tile_pool(name="wpool", bufs=1) as wpool, \
                 tc.tile_pool(name="xpool", bufs=2) as xpool, \
                 tc.tile_pool(name="ps1", bufs=6, space="PSUM") as ps1, \
                 tc.tile_pool(name="misc1", bufs=1) as misc1:
                wq_sb = wpool.tile([128, HT, OPC], F32R, tag="wq")
                wk_sb = wpool.tile([128, HT, OPC], F32R, tag="wk")
                wv_sb = wpool.tile([128, HT, OPC], F32R, tag="wv")
                # Fine-grained startup DMAs alternating across the two
                # HWDGE queues so the first matmul's inputs land fast.
                xchs = {}
                xchs[0] = xpool.tile([128, HT, IC1], F32R, tag="xch",
                                     name="xch")
                for ht in range(HT):
                    eng = nc.sync if ht % 2 == 0 else nc.scalar
                    eng.dma_start(
                        out=wq_sb[:, ht, :],
                        in_=wq[ht * 128:(ht + 1) * 128, :])
                    eng = nc.scalar if ht % 2 == 0 else nc.sync
                    eng.dma_start(
                        out=xchs[0][:, ht, :],
                        in_=xt[ht * 128:(ht + 1) * 128, 0:IC1])
                for i in range(4):
                    qt4 = HT // 4
                    nc.sync.dma_start(
                        out=wk_sb[:, i * qt4:(i + 1) * qt4, :],
                        in_=wk[:, :].rearrange("(q t p) f -> q p t f",
                                               q=4, p=128)[i])
                    nc.scalar.dma_start(
                        out=wv_sb[:, i * qt4:(i + 1) * qt4, :],
                        in_=wv[:, :].rearrange("(q t p) f -> q p t f",
                                               q=4, p=128)[i])

                ones_f = misc1.tile([128, 128], F32)
                nc.vector.memset(ones_f[:, :], 1.0)
                nc.vector.tensor_copy(ones_sb[:, :], ones_f[:, :])

                for ic in range(N_IC1):
                    if ic not in xchs:
                        xchs[ic] = xpool.tile([128, HT, IC1], F32R,
                                              tag="xch", name="xch")
                        nc.sync.dma_start(
                            out=xchs[ic][:, :, :],
                            in_=xt[:, ic * IC1:(ic + 1) * IC1]
                            .rearrange("(t p) f -> p t f", p=128))
                    xch = xchs.pop(ic)
                    # q^T and k^T: (o_local x i), stationary = W^T h-tiles
                    ncopy = 0
                    for wsb, dest in ((wq_sb, qt_sb), (wk_sb, kt_sb)):
                        for ot in range(HPC):
                            ps = ps1.tile([128, IC1], F32, tag="ps")
                            for ht in range(HT):
                                nc.tensor.matmul(
                                    ps[:, :],
                                    wsb[:, ht, ot * 128:(ot + 1) * 128],
                                    xch[:, ht, :],
                                    start=(ht == 0), stop=(ht == HT - 1))
                            if ncopy % 2 == 0:
                                nc.scalar.copy(
                                    dest[:, ot, ic * IC1:(ic + 1) * IC1],
                                    ps[:, :])
                            else:
                                nc.vector.tensor_copy(
                                    dest[:, ot, ic * IC1:(ic + 1) * IC1],
                                    ps[:, :])
                            ncopy += 1
                    # v in natural (j x o) layout, stationary = x^T tiles
                    for it in range(IC1 // 128):
                        ps = ps1.tile([128, OPC], F32, tag="ps")
                        for ht in range(HT):
                            nc.tensor.matmul(
                                ps[:, :],
                                xch[:, ht, it * 128:(it + 1) * 128],
                                wv_sb[:, ht, :],
                                start=(ht == 0), stop=(ht == HT - 1))
                        if it % 2 == 0:
                            nc.scalar.copy(
                                v_sb[:, ic * (IC1 // 128) + it, :], ps[:, :])
                        else:
                            nc.vector.tensor_copy(
                                v_sb[:, ic * (IC1 // 128) + it, :], ps[:, :])

            # ---------- phase 2: attention + pipelined o-projection ----------
            with tc.tile_pool(name="wo_pool", bufs=1) as wo_pool, \
                 tc.tile_pool(name="exp_pool", bufs=4) as exp_pool, \
                 tc.tile_pool(name="sm_pool", bufs=2) as sm_pool, \
                 tc.tile_pool(name="mst_pool", bufs=2) as mst_pool, \
                 tc.tile_pool(name="oc_pool", bufs=4) as oc_pool, \
                 tc.tile_pool(name="scps", bufs=2, space="PSUM") as scps, \
                 tc.tile_pool(name="mxps", bufs=2, space="PSUM") as mxps, \
                 tc.tile_pool(name="rsps", bufs=2, space="PSUM") as rsps, \
                 tc.tile_pool(name="ops", bufs=2, space="PSUM") as ops:
                wo_sb = wo_pool.tile([128, HPC, H], F32R, tag="wo")
                nc.scalar.dma_start(
                    out=wo_sb[:, :, :],
                    in_=wo[:, :].rearrange("(t p) f -> p t f", p=128))

                def emit_oproj(mst, gio):
                    for ot in range(H // 128):
                        op = ops.tile([128, IC2], F32, tag="op")
                        for hh in range(HPC):
                            nc.tensor.matmul(
                                op[:, :],
                                wo_sb[:, hh, ot * 128:(ot + 1) * 128],
                                mst[:, hh, :],
                                start=(hh == 0), stop=(hh == HPC - 1))
                        oc = oc_pool.tile([128, IC2], F32, tag="oc")
                        nc.vector.tensor_copy(oc[:, :], op[:, :])
                        eng = nc.sync if ot % 2 == 0 else nc.scalar
                        eng.dma_start(
                            out=out[ot * 128:(ot + 1) * 128, gio:gio + IC2],
                            in_=oc[:, :])

                pending = None
                for b in range(B):
                    for ic in range(N_IC2):
                        gio = b * L + ic * IC2
                        njt = 4 * ic + 4      # causal: j-tiles 0..4ic+3
                        mst = mst_pool.tile([128, HPC, IC2], F32R, tag="mst")
                        for h in range(HPC):
                            mx = mxps.tile([128, IC2], F32, tag="mx")
                            rs = rsps.tile([1, IC2], F32, tag="rs")
                            for jt in range(njt):
                                # live i-columns: i >= j on diagonal tiles;
                                # keep width >= 256 for fp32r full rate
                                f0 = min(max(0, 128 * jt - IC2 * ic), IC2 - 256)
                                w = IC2 - f0
                                sc = scps.tile([128, IC2], F32, tag="sc")
                                nc.tensor.matmul(
                                    sc[:, f0:],
                                    kt_sb[:, h, b * L + jt * 128:
                                          b * L + (jt + 1) * 128],
                                    qt_sb[:, h, gio + f0:gio + IC2],
                                    start=True, stop=True)
                                ex = exp_pool.tile([128, IC2], F32R, tag="ex")
                                nc.scalar.activation(
                                    ex[:, f0:], sc[:, f0:],
                                    mybir.ActivationFunctionType.Exp,
                                    scale=SCALE)
                                if jt >= 4 * ic:
                                    # zero where j > i
                                    nc.gpsimd.affine_select(
                                        ex[:, f0:], ex[:, f0:],
                                        pattern=[[1, w]],
                                        compare_op=mybir.AluOpType.is_ge,
                                        fill=0.0,
                                        base=f0 - (128 * jt - IC2 * ic),
                                        channel_multiplier=-1)
                                nc.tensor.matmul(
                                    rs[:, f0:], ones_sb[:, 0:1], ex[:, f0:],
                                    start=(jt == 0), stop=(jt == njt - 1))
                                nc.tensor.matmul(
                                    mx[:, f0:],
                                    v_sb[:, b * JT + jt,
                                         h * 128:(h + 1) * 128],
                                    ex[:, f0:],
                                    start=(jt == 0), stop=(jt == njt - 1))
                            rec_row = sm_pool.tile([1, IC2], F32, tag="recrow")
                            nc.vector.reciprocal_approx_fast(
                                out=rec_row[:, :], in_=rs[0:1, :])
                            rec_sb = sm_pool.tile([128, IC2], F32, tag="recb")
                            nc.gpsimd.partition_broadcast(
                                rec_sb[:, :], rec_row[:, :], channels=128)
                            nc.vector.tensor_mul(mst[:, h, :], mx[:, :],
                                                 rec_sb[:, :])
                        if pending is not None:
                            emit_oproj(*pending)
                        pending = (mst, gio)
                emit_oproj(*pending)
    nc.finalize()
    return nc


_NC_CACHE = None


def _get_nc():
    global _NC_CACHE
    if _NC_CACHE is None:
        _NC_CACHE = _build()
    return _NC_CACHE


def _enable_profiling():
    """Wire the axon NTFF profile hook (missing antenv.axon_hooks shim)."""
    import types
    import antenv
    if "antenv.axon_hooks" not in sys.modules:
        shim = types.ModuleType("antenv.axon_hooks")

        def set_axon_ntff_profile_hook(h):
            shim._the_hook = h

        def get_axon_ntff_profile_hook():
            return getattr(shim, "_the_hook", None)

        shim.set_axon_ntff_profile_hook = set_axon_ntff_profile_hook
        shim.get_axon_ntff_profile_hook = get_axon_ntff_profile_hook
        sys.modules["antenv.axon_hooks"] = shim
        antenv.axon_hooks = shim
    from trn_agent_boot.trn_boot import _ntff_profile_via_ctypes
    hook = _ntff_profile_via_ctypes("/opt/axon/libaxon_pjrt.so")
    sys.modules["antenv.axon_hooks"].set_axon_ntff_profile_hook(hook)
    import concourse.bass_utils as bu
    bu.upload_artifacts = lambda tmpdir: "local://" + tmpdir


def kernel(x, padding_mask, Wq, Wk, Wv, Wo):
    global LAST_EXEC_NS
    x = np.asarray(x, dtype=np.float32)
    Wq = np.asarray(Wq, dtype=np.float32)
    Wk = np.asarray(Wk, dtype=np.float32)
    Wv = np.asarray(Wv, dtype=np.float32)
    Wo = np.asarray(Wo, dtype=np.float32)

    xt = _round_fp32r(x.reshape(BL, H).T)        # (H, BL)
    wqt = _round_fp32r(Wq.T)                     # (H, H): [h, o]
    wkt = _round_fp32r(Wk.T)
    wvt = _round_fp32r(Wv.T)
    wot = _round_fp32r(Wo.T)                     # (H, H): [h_in, o]

    in_maps = []
    for c in range(N_CORES):
        sl = slice(c * OPC, (c + 1) * OPC)
        in_maps.append({
            "xt": xt,
            "wq": np.ascontiguousarray(wqt[:, sl]),
            "wk": np.ascontiguousarray(wkt[:, sl]),
            "wv": np.ascontiguousarray(wvt[:, sl]),
            "wo": np.ascontiguousarray(wot[sl, :]),
        })

    profile = os.environ.get("KERNEL_PROFILE", "0") == "1"
    if profile:
        _enable_profiling()

    nc = _get_nc()
    res = run_bass_kernel_spmd(nc, in_maps, core_ids=list(range(N_CORES)),
                               trace=profile)
    LAST_EXEC_NS = res.exec_time_ns

    total = np.zeros((H, BL), dtype=np.float64)
    for c in range(N_CORES):
        total += res.results[c]["out"]
    return np.ascontiguousarray(total.T).astype(np.float32).reshape(B, L, H)
